# revision 4
# baseline (speedup 1.0000x reference)
"""Distributed causal self-attention for 8 TRN2 NeuronCores.

Sharding: core c = (b, g) with b = c // 4 (batch), g = c % 4 (group of 4
heads).  Each core computes qkv projections for its 4 heads on x[b], the
causal attention for those heads, and a partial output projection
y_part = O_local @ W_out[local_rows].  Host sums the 4 partials per batch
element and reassembles k/v from the per-core projection outputs.

On-device layout is fully transposed (features on partitions) so no
transposes are ever needed:
  - Q^T, K^T: [feat, T] from matmul(lhsT=W[C, feat], rhs=x^T[C, T])
  - V: natural [T, feat] from matmul(lhsT=x^T[C, t-tile], rhs=Wv[C, feat])
  - scores^T tile: [tk, tq] = matmul(lhsT=K^T[d, tk], rhs=Q^T[d, tq])
  - exp on ScalarE, causal masking via rectangle narrowing + tri mask
  - O^T[d+1, tq] = matmul(lhsT=V_aug[tk, 65], rhs=expS[tk, tq]); the
    65th (ones) column of V_aug accumulates the softmax denominator free
  - y tile = matmul(lhsT=O^T[feat, tq-tile], rhs=W_out[feat, c-chunk])
"""

import numpy as np
import ml_dtypes

T = 2048
C = 1024
H = 16
DH = 64
HL = 4            # heads per core
P = 128
NQK = 512         # local q+k feature width (256 q | 256 k)
NV = 256          # local v feature width
TQC = 512         # tq chunk width
N_TQC = T // TQC  # 4
N_TT = T // P     # 16
KO = C // P       # 8

_CACHE = {}


def _build_nc():
    import concourse.tile as tile
    from concourse import bacc, mybir

    dt = mybir.dt
    f32, bf16 = dt.float32, dt.bfloat16
    AF = mybir.ActivationFunctionType
    OP = mybir.AluOpType

    nc = bacc.Bacc(None, target_bir_lowering=False)

    xt = nc.dram_tensor("xt", [C, T], bf16, kind="ExternalInput")
    wqk = nc.dram_tensor("wqk", [C, NQK], bf16, kind="ExternalInput")
    wv = nc.dram_tensor("wv", [C, NV], bf16, kind="ExternalInput")
    wo = nc.dram_tensor("wo", [NV, C], bf16, kind="ExternalInput")
    bqk = nc.dram_tensor("bqk", [P, 4], f32, kind="ExternalInput")
    bv = nc.dram_tensor("bv", [1, NV], f32, kind="ExternalInput")

    yp = nc.dram_tensor("yp", [T, C], f32, kind="ExternalOutput")
    kt = nc.dram_tensor("kt", [NV, T], f32, kind="ExternalOutput")
    vo = nc.dram_tensor("vo", [T, NV], f32, kind="ExternalOutput")

    tri_np = (np.arange(TQC)[None, :] >= np.arange(P)[:, None]).astype(
        ml_dtypes.bfloat16
    )
    tri_dram = nc.inline_tensor(tri_np, name="tri")

    with tile.TileContext(nc) as tc:
        with (
            tc.tile_pool(name="const", bufs=1) as const,
            tc.tile_pool(name="ps", bufs=2, space="PSUM") as ps_pool,
            tc.tile_pool(name="po", bufs=2, space="PSUM") as po_pool,
            tc.tile_pool(name="py", bufs=2, space="PSUM") as py_pool,
            tc.tile_pool(name="work", bufs=3) as work,
            tc.tile_pool(name="expb", bufs=3) as expb,
            tc.tile_pool(name="ot", bufs=2) as ot_pool,
            tc.tile_pool(name="sm", bufs=4) as sm,
        ):
            xt_sb = const.tile([P, KO, T], bf16)
            wqk_sb = const.tile([P, KO, NQK], bf16)
            wv_sb = const.tile([P, KO, NV], bf16)
            wo_sb = const.tile([P, 2, C], bf16)
            qT_sb = const.tile([P, 2, T], bf16)
            kT_sb = const.tile([P, 2, T], bf16)
            v_sb = const.tile([P, N_TT, HL, DH + 1], bf16)
            tri_sb = const.tile([P, TQC], bf16)
            bqk_sb = const.tile([P, 4], f32)
            bv_sb = const.tile([P, NV], f32)

            xt_r = xt[:].rearrange("(ko p) t -> ko p t", p=P)
            wqk_r = wqk[:].rearrange("(ko p) n -> ko p n", p=P)
            wv_r = wv[:].rearrange("(ko p) n -> ko p n", p=P)
            wo_r = wo[:].rearrange("(q p) n -> q p n", p=P)
            for ko in range(KO):
                nc.sync.dma_start(xt_sb[:, ko], xt_r[ko])
                nc.sync.dma_start(wqk_sb[:, ko], wqk_r[ko])
                nc.sync.dma_start(wv_sb[:, ko], wv_r[ko])
            for q in range(2):
                nc.sync.dma_start(wo_sb[:, q], wo_r[q])
            nc.sync.dma_start(tri_sb[:], tri_dram[:])
            nc.sync.dma_start(bqk_sb[:], bqk[:])
            nc.sync.dma_start(bv_sb[:], bv[:].to_broadcast((P, NV)))
            nc.vector.memset(v_sb[:, :, :, DH : DH + 1], 1.0)

            # ---- Phase 1a: Q^T / K^T blocks (features on partitions) ----
            for bb in range(4):
                for ti in range(N_TQC):
                    tsl = slice(ti * TQC, (ti + 1) * TQC)
                    ps = ps_pool.tile([P, 2 * TQC], f32, tag="ps", name="ps")[:, :TQC]
                    for ko in range(KO):
                        nc.tensor.matmul(
                            ps,
                            wqk_sb[:, ko, bb * P : (bb + 1) * P],
                            xt_sb[:, ko, tsl],
                            start=(ko == 0),
                            stop=(ko == KO - 1),
                        )
                    if bb < 2:
                        # q: (psum + bias) * 1/sqrt(DH), cast to bf16
                        nc.vector.tensor_scalar(
                            qT_sb[:, bb, tsl],
                            ps,
                            bqk_sb[:, bb : bb + 1],
                            0.125,
                            OP.add,
                            OP.mult,
                        )
                    else:
                        kf = work.tile([P, TQC], f32, tag="work", name="wk")
                        nc.vector.tensor_scalar_add(
                            kf, ps, bqk_sb[:, bb : bb + 1]
                        )
                        nc.sync.dma_start(
                            kt[(bb - 2) * P : (bb - 1) * P, tsl], kf
                        )
                        nc.scalar.copy(kT_sb[:, bb - 2, tsl], kf)

            # ---- Phase 1b: V (natural layout, t on partitions) ----
            for tt in range(N_TT):
                psl = slice(tt * P, (tt + 1) * P)
                ps = ps_pool.tile([P, 2 * TQC], f32, tag="ps", name="ps")[:, :NV]
                for ko in range(KO):
                    nc.tensor.matmul(
                        ps,
                        xt_sb[:, ko, psl],
                        wv_sb[:, ko],
                        start=(ko == 0),
                        stop=(ko == KO - 1),
                    )
                vf = work.tile([P, TQC], f32, tag="work", name="wk")[:, :NV]
                nc.vector.tensor_add(vf, ps, bv_sb[:, :NV])
                nc.sync.dma_start(vo[psl], vf)
                nc.scalar.copy(
                    v_sb[:, tt, :, 0:DH], vf.rearrange("p (h d) -> p h d", h=HL)
                )

            # ---- Phase 2: attention + Phase 3: out-projection ----
            for c in range(N_TQC):
                t0 = c * TQC
                oT = ot_pool.tile([P, 2, TQC], bf16, tag="ot")
                for h in range(HL):
                    bb, off = h // 2, (h % 2) * 64
                    po = po_pool.tile([DH + 1, TQC], f32, tag="po")
                    n_below = 4 * c
                    last_d = (n_below - 1) if n_below else (4 * c + 3)

                    # band tiles first (d = 4c is full width, start=True)
                    for j in range(4):
                        d = 4 * c + j
                        k_off = j * P
                        w = TQC - k_off
                        psb = ps_pool.tile([P, 2 * TQC], f32, tag="ps", name="ps")[:, :w]
                        nc.tensor.matmul(
                            psb,
                            kT_sb[off : off + 64, bb, d * P : (d + 1) * P],
                            qT_sb[off : off + 64, bb, t0 + k_off : t0 + TQC],
                            start=True,
                            stop=True,
                        )
                        eb = expb.tile([P, 2 * TQC], bf16, tag="expb", name="eb")[:, :w]
                        nc.scalar.activation(eb, psb, AF.Exp)
                        nc.vector.tensor_mul(eb, eb, tri_sb[:, :w])
                        nc.tensor.matmul(
                            po[:, k_off:TQC],
                            v_sb[:, d, h],
                            eb,
                            start=(j == 0),
                            stop=(d == last_d),
                            skip_group_check=True,
                        )

                    # below-diagonal tiles, exp batched in pairs
                    for d0 in range(0, n_below, 2):
                        dpair = (d0, d0 + 1) if d0 + 1 < n_below else (d0,)
                        pg = ps_pool.tile([P, 2 * TQC], f32, tag="ps", name="ps")[
                            :, : len(dpair) * TQC
                        ]
                        for u, d in enumerate(dpair):
                            nc.tensor.matmul(
                                pg[:, u * TQC : (u + 1) * TQC],
                                kT_sb[off : off + 64, bb, d * P : (d + 1) * P],
                                qT_sb[off : off + 64, bb, t0 : t0 + TQC],
                                start=True,
                                stop=True,
                            )
                        eg = expb.tile([P, 2 * TQC], bf16, tag="expb", name="eb")[
                            :, : len(dpair) * TQC
                        ]
                        nc.scalar.activation(eg, pg, AF.Exp)
                        for u, d in enumerate(dpair):
                            nc.tensor.matmul(
                                po,
                                v_sb[:, d, h],
                                eg[:, u * TQC : (u + 1) * TQC],
                                start=False,
                                stop=(d == last_d),
                                skip_group_check=True,
                            )

                    # normalize: oT[head rows] = O^T * (1 / denom)
                    rc = sm.tile([1, TQC], f32, tag="rc")
                    nc.vector.reciprocal(rc, po[DH : DH + 1])
                    rcb = sm.tile([64, TQC], f32, tag="rcb")
                    nc.gpsimd.partition_broadcast(rcb[:], rc[:])
                    nc.vector.tensor_mul(
                        oT[off : off + 64, bb], po[0:DH], rcb
                    )

                # out-projection for this tq chunk
                for sub in range(TQC // P):
                    rsl = slice(t0 + sub * P, t0 + (sub + 1) * P)
                    for cc in range(C // TQC):
                        py = py_pool.tile([P, TQC], f32, tag="py")
                        for pair in range(2):
                            nc.tensor.matmul(
                                py,
                                oT[:, pair, sub * P : (sub + 1) * P],
                                wo_sb[:, pair, cc * TQC : (cc + 1) * TQC],
                                start=(pair == 0),
                                stop=(pair == 1),
                            )
                        yf = work.tile([P, TQC], f32, tag="work", name="wk")
                        nc.vector.tensor_copy(yf, py)
                        nc.sync.dma_start(
                            yp[rsl, cc * TQC : (cc + 1) * TQC], yf
                        )

    nc.compile()
    return nc


def _get_nc():
    if "nc" not in _CACHE:
        _CACHE["nc"] = _build_nc()
    return _CACHE["nc"]


def _make_in_maps(x, W_qkv, b_qkv, W_out):
    bf = ml_dtypes.bfloat16
    in_maps = []
    for core in range(8):
        b, g = divmod(core, 4)
        fs = slice(g * NV, (g + 1) * NV)
        wq = W_qkv[:, 0:C][:, fs]
        wk = W_qkv[:, C : 2 * C][:, fs]
        wv_ = W_qkv[:, 2 * C : 3 * C][:, fs]
        bq = b_qkv[0:C][fs]
        bk = b_qkv[C : 2 * C][fs]
        bv_ = b_qkv[2 * C : 3 * C][fs]
        in_maps.append(
            {
                "xt": np.ascontiguousarray(x[b].T).astype(bf),
                "wqk": np.ascontiguousarray(
                    np.concatenate([wq, wk], axis=1)
                ).astype(bf),
                "wv": np.ascontiguousarray(wv_).astype(bf),
                "wo": np.ascontiguousarray(W_out[fs, :]).astype(bf),
                "bqk": np.ascontiguousarray(
                    np.concatenate([bq, bk]).reshape(4, P).T
                ).astype(np.float32),
                "bv": np.ascontiguousarray(bv_.reshape(1, NV)).astype(
                    np.float32
                ),
            }
        )
    return in_maps


def _gather(results, b_out):
    y = np.zeros((2, T, C), np.float32)
    k = np.zeros((2, H, T, DH), np.float32)
    v = np.zeros((2, H, T, DH), np.float32)
    for core in range(8):
        b, g = divmod(core, 4)
        r = results[core]
        y[b] += np.asarray(r["yp"], np.float32)
        k[b, g * HL : (g + 1) * HL] = (
            np.asarray(r["kt"], np.float32).reshape(HL, DH, T).transpose(0, 2, 1)
        )
        v[b, g * HL : (g + 1) * HL] = (
            np.asarray(r["vo"], np.float32).reshape(T, HL, DH).transpose(1, 0, 2)
        )
    y += np.asarray(b_out, np.float32)
    return y, k, v


def _run(inputs, **spmd_kwargs):
    from concourse.bass_utils import run_bass_kernel_spmd

    x = np.asarray(inputs["x"], np.float32)
    W_qkv = np.asarray(inputs["W_qkv"], np.float32)
    b_qkv = np.asarray(inputs["b_qkv"], np.float32)
    W_out = np.asarray(inputs["W_out"], np.float32)
    b_out = np.asarray(inputs["b_out"], np.float32)

    nc = _get_nc()
    in_maps = _make_in_maps(x, W_qkv, b_qkv, W_out)
    res = run_bass_kernel_spmd(nc, in_maps, core_ids=list(range(8)), **spmd_kwargs)
    return _gather(res.results, b_out), res


def kernel(**inputs):
    (y, k, v), _ = _run(inputs)
    return y, k, v


def _install_ntff_hook():
    """The agent image's antenv lacks axon_hooks; synthesize it so
    run_bass_kernel_spmd(trace=True) can capture NTFF profiles."""
    import sys
    import types

    if "antenv.axon_hooks" in sys.modules:
        return
    mod = types.ModuleType("antenv.axon_hooks")
    mod._hook = None

    def set_axon_ntff_profile_hook(h):
        mod._hook = h

    def get_axon_ntff_profile_hook():
        return mod._hook

    mod.set_axon_ntff_profile_hook = set_axon_ntff_profile_hook
    mod.get_axon_ntff_profile_hook = get_axon_ntff_profile_hook
    sys.modules["antenv.axon_hooks"] = mod
    import antenv

    antenv.axon_hooks = mod
    try:
        from trn_agent_boot.trn_boot import _ntff_profile_via_ctypes

        mod._hook = _ntff_profile_via_ctypes("/opt/axon/libaxon_pjrt.so")
    except Exception as e:
        print(f"NTFF hook install failed: {e}")


def kernel_profiled(**inputs):
    """Like kernel() but with NTFF tracing; returns ((y, k, v), exec_time_ns, res)."""
    _install_ntff_hook()
    out, res = _run(inputs, trace=True)
    return out, res.exec_time_ns, res


# revision 10
# speedup vs baseline: 1.4373x; 1.4373x over previous
"""Distributed causal self-attention for 8 TRN2 NeuronCores.

Sharding: core c = (b, g) with b = c // 4 (batch), g = c % 4 (group of 4
heads).  Each core computes qkv projections for its 4 heads on x[b], the
causal attention for those heads, and a partial output projection
y_part = O_local @ W_out[local_rows].  Host sums the 4 partials per batch
element and reassembles k/v from the per-core projection outputs.

On-device layout is fully transposed (features on partitions) so no
transposes are ever needed:
  - Q^T, K^T: [feat, T] from matmul(lhsT=W[C, feat], rhs=x^T[C, T])
  - V: natural [T, feat] from matmul(lhsT=x^T[C, t-tile], rhs=Wv[C, feat])
  - scores^T tile: [tk, tq] = matmul(lhsT=K^T[d, tk], rhs=Q^T[d, tq])
  - exp on ScalarE, causal masking via rectangle narrowing + tri mask
  - O^T[d+1, tq] = matmul(lhsT=V_aug[tk, 65], rhs=expS[tk, tq]); the
    65th (ones) column of V_aug accumulates the softmax denominator free
  - y tile = matmul(lhsT=O^T[feat, tq-tile], rhs=W_out[feat, c-chunk])
"""

import numpy as np
import ml_dtypes

T = 2048
C = 1024
H = 16
DH = 64
HL = 4            # heads per core
P = 128
NQK = 512         # local q+k feature width (256 q | 256 k)
NV = 256          # local v feature width
TQC = 512         # tq chunk width
N_TQC = T // TQC  # 4
N_TT = T // P     # 16
KO = C // P       # 8

_CACHE = {}


def _build_nc():
    import concourse.tile as tile
    from concourse import bacc, mybir

    dt = mybir.dt
    f32, bf16 = dt.float32, dt.bfloat16
    AF = mybir.ActivationFunctionType
    OP = mybir.AluOpType

    nc = bacc.Bacc(None, target_bir_lowering=False)

    xt = nc.dram_tensor("xt", [C, T], bf16, kind="ExternalInput")
    wqk = nc.dram_tensor("wqk", [C, NQK], bf16, kind="ExternalInput")
    wv = nc.dram_tensor("wv", [C, NV], bf16, kind="ExternalInput")
    wo = nc.dram_tensor("wo", [NV, C], bf16, kind="ExternalInput")
    bqk = nc.dram_tensor("bqk", [P, 4], f32, kind="ExternalInput")
    bv = nc.dram_tensor("bv", [1, NV], f32, kind="ExternalInput")

    yp = nc.dram_tensor("yp", [T, C], f32, kind="ExternalOutput")
    kt = nc.dram_tensor("kt", [NV, T], f32, kind="ExternalOutput")
    vo = nc.dram_tensor("vo", [T, NV], f32, kind="ExternalOutput")

    tri_np = (np.arange(TQC)[None, :] >= np.arange(P)[:, None]).astype(
        ml_dtypes.bfloat16
    )
    tri_dram = nc.inline_tensor(tri_np, name="tri")

    with tile.TileContext(nc) as tc:
        with (
            tc.tile_pool(name="const", bufs=1) as const,
            tc.tile_pool(name="ps", bufs=2, space="PSUM") as ps_pool,
            tc.tile_pool(name="po", bufs=2, space="PSUM") as po_pool,
            tc.tile_pool(name="py", bufs=2, space="PSUM") as py_pool,
            tc.tile_pool(name="work", bufs=3) as work,
            tc.tile_pool(name="expb", bufs=4) as expb,
            tc.tile_pool(name="ot", bufs=2) as ot_pool,
            tc.tile_pool(name="sm", bufs=4) as sm,
        ):
            xt_sb = const.tile([P, KO, T], bf16)
            wqk_sb = const.tile([P, KO, NQK], bf16)
            wv_sb = const.tile([P, KO, NV], bf16)
            wo_sb = const.tile([P, 2, C], bf16)
            qT_sb = const.tile([P, 2, T], bf16)
            kT_sb = const.tile([P, 2, T], bf16)
            v_sb = const.tile([P, N_TT, HL, DH + 1], bf16)
            tri_sb = const.tile([P, TQC], bf16)
            bqk_sb = const.tile([P, 4], f32)
            bv_sb = const.tile([P, NV], f32)

            xt_r = xt[:].rearrange("(ko p) t -> ko p t", p=P)
            wqk_r = wqk[:].rearrange("(ko p) n -> ko p n", p=P)
            wv_r = wv[:].rearrange("(ko p) n -> ko p n", p=P)
            wo_r = wo[:].rearrange("(q p) n -> q p n", p=P)
            for ko in range(KO):
                nc.sync.dma_start(xt_sb[:, ko], xt_r[ko])
                nc.sync.dma_start(wqk_sb[:, ko], wqk_r[ko])
                nc.sync.dma_start(wv_sb[:, ko], wv_r[ko])
            for q in range(2):
                nc.sync.dma_start(wo_sb[:, q], wo_r[q])
            nc.sync.dma_start(tri_sb[:], tri_dram[:])
            nc.sync.dma_start(bqk_sb[:], bqk[:])
            nc.sync.dma_start(bv_sb[:], bv[:].to_broadcast((P, NV)))
            nc.vector.memset(v_sb[:, :, :, DH : DH + 1], 1.0)

            # ---- Phase 1a: Q^T / K^T blocks (features on partitions) ----
            for bb in range(4):
                for ti in range(N_TQC):
                    tsl = slice(ti * TQC, (ti + 1) * TQC)
                    ps = ps_pool.tile([P, 2 * TQC], f32, tag="ps", name="ps")[:, :TQC]
                    for ko in range(KO):
                        nc.tensor.matmul(
                            ps,
                            wqk_sb[:, ko, bb * P : (bb + 1) * P],
                            xt_sb[:, ko, tsl],
                            start=(ko == 0),
                            stop=(ko == KO - 1),
                        )
                    if bb < 2:
                        # q: (psum + bias) * 1/sqrt(DH), cast to bf16
                        nc.vector.tensor_scalar(
                            qT_sb[:, bb, tsl],
                            ps,
                            bqk_sb[:, bb : bb + 1],
                            0.125,
                            OP.add,
                            OP.mult,
                        )
                    else:
                        kf = work.tile([P, TQC], f32, tag="work", name="wk")
                        nc.vector.tensor_scalar_add(
                            kf, ps, bqk_sb[:, bb : bb + 1]
                        )
                        nc.sync.dma_start(
                            kt[(bb - 2) * P : (bb - 1) * P, tsl], kf
                        )
                        nc.scalar.copy(kT_sb[:, bb - 2, tsl], kf)

            # ---- Phase 1b: V (natural layout, t on partitions) ----
            for tt in range(N_TT):
                psl = slice(tt * P, (tt + 1) * P)
                ps = ps_pool.tile([P, 2 * TQC], f32, tag="ps", name="ps")[:, :NV]
                for ko in range(KO):
                    nc.tensor.matmul(
                        ps,
                        xt_sb[:, ko, psl],
                        wv_sb[:, ko],
                        start=(ko == 0),
                        stop=(ko == KO - 1),
                    )
                vf = work.tile([P, TQC], f32, tag="work", name="wk")[:, :NV]
                nc.vector.tensor_add(vf, ps, bv_sb[:, :NV])
                nc.sync.dma_start(vo[psl], vf)
                nc.scalar.copy(
                    v_sb[:, tt, :, 0:DH], vf.rearrange("p (h d) -> p h d", h=HL)
                )

            # ---- Phase 2: attention + Phase 3: out-projection ----
            for c in range(N_TQC):
                t0 = c * TQC
                oT = ot_pool.tile([P, 2, TQC], bf16, tag="ot")
                for h in range(HL):
                    bb, off = h // 2, (h % 2) * 64
                    po = po_pool.tile([DH + 1, TQC], f32, tag="po")
                    n_below = 4 * c
                    last_d = (n_below - 1) if n_below else (4 * c + 3)

                    # band tiles: packed in pairs so one exp covers two tiles
                    # pair A: d=4c (w=512, start=True) + d=4c+1 (w=384)
                    # pair B: d=4c+2 (w=256) + d=4c+3 (w=128)
                    for pair in range(2):
                        js = (0, 1) if pair == 0 else (2, 3)
                        offs = []  # (j, d, k_off, w, slot_off)
                        so = 0
                        for j in js:
                            d = 4 * c + j
                            k_off = j * P
                            w = TQC - k_off
                            offs.append((j, d, k_off, w, so))
                            so += w
                        pb = ps_pool.tile([P, 2 * TQC], f32, tag="ps", name="ps")
                        for j, d, k_off, w, so in offs:
                            nc.tensor.matmul(
                                pb[:, so : so + w],
                                kT_sb[off : off + 64, bb, d * P : (d + 1) * P],
                                qT_sb[off : off + 64, bb, t0 + k_off : t0 + TQC],
                                start=True,
                                stop=True,
                            )
                        tot = sum(o[3] for o in offs)
                        eb = expb.tile([P, 2 * TQC], bf16, tag="expb", name="eb")
                        nc.scalar.activation(eb[:, :tot], pb[:, :tot], AF.Exp)
                        for j, d, k_off, w, so in offs:
                            nc.vector.tensor_mul(
                                eb[:, so : so + w], eb[:, so : so + w], tri_sb[:, :w]
                            )
                            nc.tensor.matmul(
                                po[:, k_off:TQC],
                                v_sb[:, d, h],
                                eb[:, so : so + w],
                                start=(j == 0),
                                stop=(d == last_d),
                                skip_group_check=True,
                            )

                    # below-diagonal tiles, exp batched in pairs
                    for d0 in range(0, n_below, 2):
                        dpair = (d0, d0 + 1) if d0 + 1 < n_below else (d0,)
                        pg = ps_pool.tile([P, 2 * TQC], f32, tag="ps", name="ps")[
                            :, : len(dpair) * TQC
                        ]
                        for u, d in enumerate(dpair):
                            nc.tensor.matmul(
                                pg[:, u * TQC : (u + 1) * TQC],
                                kT_sb[off : off + 64, bb, d * P : (d + 1) * P],
                                qT_sb[off : off + 64, bb, t0 : t0 + TQC],
                                start=True,
                                stop=True,
                            )
                        eg = expb.tile([P, 2 * TQC], bf16, tag="expb", name="eb")[
                            :, : len(dpair) * TQC
                        ]
                        nc.scalar.activation(eg, pg, AF.Exp)
                        for u, d in enumerate(dpair):
                            nc.tensor.matmul(
                                po,
                                v_sb[:, d, h],
                                eg[:, u * TQC : (u + 1) * TQC],
                                start=False,
                                stop=(d == last_d),
                                skip_group_check=True,
                            )

                    # normalize: oT[head rows] = O^T * (1 / denom)
                    rc = sm.tile([1, TQC], f32, tag="rc")
                    rs = sm.tile([1, TQC], f32, tag="rs")
                    nc.vector.reciprocal_approx_accurate(rc, po[DH : DH + 1], rs)
                    rcb = sm.tile([64, TQC], f32, tag="rcb")
                    nc.gpsimd.partition_broadcast(rcb[:], rc[:])
                    nc.vector.tensor_mul(
                        oT[off : off + 64, bb], po[0:DH], rcb
                    )

                # out-projection for this tq chunk
                for sub in range(TQC // P):
                    rsl = slice(t0 + sub * P, t0 + (sub + 1) * P)
                    for cc in range(C // TQC):
                        py = py_pool.tile([P, TQC], f32, tag="py")
                        for pair in range(2):
                            nc.tensor.matmul(
                                py,
                                oT[:, pair, sub * P : (sub + 1) * P],
                                wo_sb[:, pair, cc * TQC : (cc + 1) * TQC],
                                start=(pair == 0),
                                stop=(pair == 1),
                            )
                        yf = work.tile([P, TQC], f32, tag="work", name="wk")
                        nc.vector.tensor_copy(yf, py)
                        nc.sync.dma_start(
                            yp[rsl, cc * TQC : (cc + 1) * TQC], yf
                        )

    nc.compile()
    return nc


def _get_nc():
    if "nc" not in _CACHE:
        _CACHE["nc"] = _build_nc()
    return _CACHE["nc"]


def _make_in_maps(x, W_qkv, b_qkv, W_out):
    bf = ml_dtypes.bfloat16
    in_maps = []
    for core in range(8):
        b, g = divmod(core, 4)
        fs = slice(g * NV, (g + 1) * NV)
        wq = W_qkv[:, 0:C][:, fs]
        wk = W_qkv[:, C : 2 * C][:, fs]
        wv_ = W_qkv[:, 2 * C : 3 * C][:, fs]
        bq = b_qkv[0:C][fs]
        bk = b_qkv[C : 2 * C][fs]
        bv_ = b_qkv[2 * C : 3 * C][fs]
        in_maps.append(
            {
                "xt": np.ascontiguousarray(x[b].T).astype(bf),
                "wqk": np.ascontiguousarray(
                    np.concatenate([wq, wk], axis=1)
                ).astype(bf),
                "wv": np.ascontiguousarray(wv_).astype(bf),
                "wo": np.ascontiguousarray(W_out[fs, :]).astype(bf),
                "bqk": np.ascontiguousarray(
                    np.concatenate([bq, bk]).reshape(4, P).T
                ).astype(np.float32),
                "bv": np.ascontiguousarray(bv_.reshape(1, NV)).astype(
                    np.float32
                ),
            }
        )
    return in_maps


def _gather(results, b_out):
    y = np.zeros((2, T, C), np.float32)
    k = np.zeros((2, H, T, DH), np.float32)
    v = np.zeros((2, H, T, DH), np.float32)
    for core in range(8):
        b, g = divmod(core, 4)
        r = results[core]
        y[b] += np.asarray(r["yp"], np.float32)
        k[b, g * HL : (g + 1) * HL] = (
            np.asarray(r["kt"], np.float32).reshape(HL, DH, T).transpose(0, 2, 1)
        )
        v[b, g * HL : (g + 1) * HL] = (
            np.asarray(r["vo"], np.float32).reshape(T, HL, DH).transpose(1, 0, 2)
        )
    y += np.asarray(b_out, np.float32)
    return y, k, v


def _run(inputs, **spmd_kwargs):
    from concourse.bass_utils import run_bass_kernel_spmd

    x = np.asarray(inputs["x"], np.float32)
    W_qkv = np.asarray(inputs["W_qkv"], np.float32)
    b_qkv = np.asarray(inputs["b_qkv"], np.float32)
    W_out = np.asarray(inputs["W_out"], np.float32)
    b_out = np.asarray(inputs["b_out"], np.float32)

    nc = _get_nc()
    in_maps = _make_in_maps(x, W_qkv, b_qkv, W_out)
    res = run_bass_kernel_spmd(nc, in_maps, core_ids=list(range(8)), **spmd_kwargs)
    return _gather(res.results, b_out), res


def kernel(**inputs):
    (y, k, v), _ = _run(inputs)
    return y, k, v


def _install_ntff_hook():
    """The agent image's antenv lacks axon_hooks; synthesize it so
    run_bass_kernel_spmd(trace=True) can capture NTFF profiles."""
    import sys
    import types

    if "antenv.axon_hooks" in sys.modules:
        return
    mod = types.ModuleType("antenv.axon_hooks")
    mod._hook = None

    def set_axon_ntff_profile_hook(h):
        mod._hook = h

    def get_axon_ntff_profile_hook():
        return mod._hook

    mod.set_axon_ntff_profile_hook = set_axon_ntff_profile_hook
    mod.get_axon_ntff_profile_hook = get_axon_ntff_profile_hook
    sys.modules["antenv.axon_hooks"] = mod
    import antenv

    antenv.axon_hooks = mod
    try:
        from trn_agent_boot.trn_boot import _ntff_profile_via_ctypes

        mod._hook = _ntff_profile_via_ctypes("/opt/axon/libaxon_pjrt.so")
    except Exception as e:
        print(f"NTFF hook install failed: {e}")


def kernel_profiled(**inputs):
    """Like kernel() but with NTFF tracing; returns ((y, k, v), exec_time_ns, res)."""
    _install_ntff_hook()
    out, res = _run(inputs, trace=True)
    return out, res.exec_time_ns, res
